# revision 6
# baseline (speedup 1.0000x reference)
"""Trainium2 Bass kernel for nn_Cross_AttentionHead_withMask.

Cross-attention head: q = rope(x_text @ Wq.T), k = rope2d(x_image @ Wk.T),
v = x_image @ Wv.T, out = softmax(q k^T / sqrt(512)) v.
(x_latex_mask is accepted but unused — it is dead in the reference.)

Sharding: data-parallel over batch B=8, one batch per NeuronCore (8 cores).

Per-core device program (all matmuls bf16, accumulation/softmax stats fp32):
  - host ships x_image[b].T / x_text[b].T (bf16) so the contraction dim (C)
    lands on SBUF partitions without any on-device transposes
  - head dim is permuted to evens-then-odds so RoPE pairs become the row
    blocks [0:32] / [32:64]; rope = A*CC + partner(A)*SS (2 muls + 1 add)
  - scores computed transposed: weiT[t, s] = K2[:, t-tile].T @ Q2[:, s-chunk]
  - exp on ScalarE straight out of PSUM with the 1/sqrt(512) scale fused
  - attention-out: outT[h, s] += v_aug[t-tile].T @ expT, where v_aug carries
    a ones column so row 64 accumulates the softmax denominator for free
  - epilogue: PE-transpose [65, 128] -> [128, 65], per-partition reciprocal
    of the Z column, tensor_scalar multiply, DMA out
"""
import numpy as np
from contextlib import ExitStack

import ml_dtypes

B, TQ, TK = 8, 2048, 4096
DIM_IMG, DIM_TXT, HS = 512, 128, 64
N_CORES = 8
SCALE = float(DIM_IMG) ** -0.5  # reference scales by sqrt(image embed dim)

BF16 = ml_dtypes.bfloat16

_prog_cache = {}


def _patch_tile_drain():
    """This walrus build rejects a Drain carrying >1 sem wait; split the
    TileContext exit waits onto one-wait NoOps."""
    import concourse.tile as tile
    from concourse import mybir
    from concourse.vector_clock import ScopedClock

    if getattr(tile.TileContext, "_drain_patched", False):
        return

    def _drain_and_barrier(self, tick_clock, wait_clock):
        nc = self.nc
        nop = nc.sync.nop()
        wait_clock.add_sem_waits(nop.ins, ScopedClock({None: tick_clock.global_clock}))
        si = nop.ins.sync_info
        waits = list(si.on_wait) if si is not None else []
        if len(waits) > 1:
            nop.ins.sync_info = mybir.SyncInfo(on_wait=[waits[0]], on_update=[])
            for w in waits[1:]:
                extra = nc.sync.nop()
                extra.ins.sync_info = mybir.SyncInfo(on_wait=[w], on_update=[])
        nc.sync.drain()
        nc.all_engine_barrier()
        assert self.sems is not None
        popped = nc._tile_sem_poison_stack.pop()
        assert popped is self._sem_poison
        nc.clear_and_free_semaphores(list(self.sems.allocated().values()))
        nc.all_engine_barrier()

    tile.TileContext._drain_and_barrier = _drain_and_barrier
    tile.TileContext._drain_patched = True


def _split_excess_waits(nc):
    """This walrus build caps sem waits per instruction (1 for DMA/Drain-style
    control instructions, 2 for compute). Move excess waits onto same-engine
    NoOps inserted right before the offending instruction — the engine queue
    is FIFO, so blocking dispatch on the NoOp is semantically equivalent."""
    from concourse import mybir

    ctr = 0
    for fn in nc.m.functions:
        for b in fn.blocks:
            il = b.instructions
            new = []
            changed = False
            for inst in il:
                si = inst.sync_info
                waits = list(si.on_wait) if si is not None else []
                lim = 1
                if len(waits) > lim:
                    for w in waits[lim:]:
                        nop = mybir.InstNoOp(name=f"wsplit-{ctr}", ins=[], outs=[])
                        ctr += 1
                        nop.engine = inst.engine
                        nop.sync_info = mybir.SyncInfo(on_wait=[w], on_update=[])
                        new.append(nop)
                    inst.sync_info = mybir.SyncInfo(
                        on_wait=waits[:lim], on_update=list(si.on_update)
                    )
                    changed = True
                new.append(inst)
            if changed:
                b.instructions = new


def build_program():
    """Build the single-core Bass program (same program runs SPMD on 8 cores)."""
    if "nc" in _prog_cache:
        return _prog_cache["nc"]

    _patch_tile_drain()
    import concourse.bass as bass
    import concourse.tile as tile
    from concourse import mybir
    from concourse.masks import make_identity

    FP = mybir.dt.float32
    BF = mybir.dt.bfloat16

    nc = bass.Bass("TRN2", target_bir_lowering=False, debug=False)
    xt = nc.dram_tensor("xt", [DIM_IMG, TK], BF, kind="ExternalInput").ap()
    xtt = nc.dram_tensor("xtt", [DIM_TXT, TQ], BF, kind="ExternalInput").ap()
    wk = nc.dram_tensor("wk", [DIM_IMG, HS], BF, kind="ExternalInput").ap()
    wq = nc.dram_tensor("wq", [DIM_TXT, HS], BF, kind="ExternalInput").ap()
    wv = nc.dram_tensor("wv", [DIM_IMG, HS], BF, kind="ExternalInput").ap()
    cck = nc.dram_tensor("cck", [HS, TK], BF, kind="ExternalInput").ap()
    ssk = nc.dram_tensor("ssk", [HS, TK], BF, kind="ExternalInput").ap()
    ccq = nc.dram_tensor("ccq", [HS, TQ], BF, kind="ExternalInput").ap()
    ssq = nc.dram_tensor("ssq", [HS, TQ], BF, kind="ExternalInput").ap()
    out = nc.dram_tensor("out", [TQ, HS], FP, kind="ExternalOutput").ap()

    Exp = mybir.ActivationFunctionType.Exp
    NC4 = DIM_IMG // 128  # 4 c-chunks
    NT = TK // 128  # 32 t-tiles
    NSC = TQ // 512  # 4 s-chunks

    with tile.TileContext(nc) as tc:
        with ExitStack() as ctx:
            const = ctx.enter_context(tc.tile_pool(name="const", bufs=1))

            # ---- resident loads ----
            xt_sb = []
            for ci in range(NC4):
                t = const.tile([128, TK], BF, tag=f"xt{ci}")
                nc.sync.dma_start(t[:], xt[ci * 128 : (ci + 1) * 128, :])
                xt_sb.append(t)
            xtt_sb = const.tile([128, TQ], BF, tag="xtt")
            nc.sync.dma_start(xtt_sb[:], xtt[:])
            wk_sb = const.tile([128, NC4 * HS], BF, tag="wk")
            nc.sync.dma_start(
                wk_sb[:].rearrange("p (a h) -> p a h", a=NC4),
                wk.rearrange("(a p) h -> p a h", p=128),
            )
            wv_sb = const.tile([128, NC4 * HS], BF, tag="wv")
            nc.sync.dma_start(
                wv_sb[:].rearrange("p (a h) -> p a h", a=NC4),
                wv.rearrange("(a p) h -> p a h", p=128),
            )
            wq_sb = const.tile([128, HS], BF, tag="wq")
            nc.sync.dma_start(wq_sb[:], wq[:])
            cck_sb = const.tile([HS, TK], BF, tag="cck")
            nc.sync.dma_start(cck_sb[:], cck[:])
            ssk_sb = const.tile([HS, TK], BF, tag="ssk")
            nc.sync.dma_start(ssk_sb[:], ssk[:])
            ccq_sb = const.tile([HS, TQ], BF, tag="ccq")
            nc.sync.dma_start(ccq_sb[:], ccq[:])
            ssq_sb = const.tile([HS, TQ], BF, tag="ssq")
            nc.sync.dma_start(ssq_sb[:], ssq[:])
            ident = const.tile([128, 128], FP, tag="ident")
            make_identity(nc, ident[:])

            kt_pre = const.tile([HS, TK], BF, tag="ktpre")
            qt_pre = const.tile([HS, TQ], BF, tag="qtpre")
            v_all = const.tile([128, NT * 65], BF, tag="vall")

            # ---- projections ----
            with tc.tile_pool(name="pp", bufs=2, space="PSUM") as pp:
                for j in range(TQ // 512):
                    ps = pp.tile([HS, 512], FP, tag="ps")
                    nc.tensor.matmul(
                        ps[:], lhsT=wq_sb[:], rhs=xtt_sb[:, j * 512 : (j + 1) * 512],
                        start=True, stop=True,
                    )
                    nc.scalar.copy(qt_pre[:, j * 512 : (j + 1) * 512], ps[:])
                for j in range(TK // 512):
                    ps = pp.tile([HS, 512], FP, tag="ps")
                    for ci in range(NC4):
                        nc.tensor.matmul(
                            ps[:],
                            lhsT=wk_sb[:, ci * HS : (ci + 1) * HS],
                            rhs=xt_sb[ci][:, j * 512 : (j + 1) * 512],
                            start=(ci == 0), stop=(ci == NC4 - 1),
                        )
                    nc.scalar.copy(kt_pre[:, j * 512 : (j + 1) * 512], ps[:])
                for tt in range(NT):
                    ps = pp.tile([128, HS], FP, tag="psv")
                    for ci in range(NC4):
                        nc.tensor.matmul(
                            ps[:],
                            lhsT=xt_sb[ci][:, tt * 128 : (tt + 1) * 128],
                            rhs=wv_sb[:, ci * HS : (ci + 1) * HS],
                            start=(ci == 0), stop=(ci == NC4 - 1),
                        )
                    nc.vector.tensor_copy(v_all[:, tt * 65 : tt * 65 + HS], ps[:])
            # ones column for the Z (softmax denominator) row
            nc.gpsimd.memset(v_all[:, HS :: 65], 1.0)

            # ---- RoPE (on the evens/odds-permuted transposed projections) ----
            pk = const.tile([HS, TK], BF, tag="pk")
            nc.sync.dma_start(pk[0:32, :], kt_pre[32:64, :])
            nc.sync.dma_start(pk[32:64, :], kt_pre[0:32, :])
            pq = const.tile([HS, TQ], BF, tag="pq")
            nc.sync.dma_start(pq[0:32, :], qt_pre[32:64, :])
            nc.sync.dma_start(pq[32:64, :], qt_pre[0:32, :])

            t1k = const.tile([HS, TK], BF, tag="t1k")
            nc.vector.tensor_mul(t1k[:], kt_pre[:], cck_sb[:])
            t2k = const.tile([HS, TK], BF, tag="t2k")
            nc.vector.tensor_mul(t2k[:], pk[:], ssk_sb[:])
            K2 = const.tile([HS, TK], BF, tag="K2")
            nc.vector.tensor_add(K2[:], t1k[:], t2k[:])

            t1q = const.tile([HS, TQ], BF, tag="t1q")
            nc.vector.tensor_mul(t1q[:], qt_pre[:], ccq_sb[:])
            t2q = const.tile([HS, TQ], BF, tag="t2q")
            nc.vector.tensor_mul(t2q[:], pq[:], ssq_sb[:])
            Q2 = const.tile([HS, TQ], BF, tag="Q2")
            nc.vector.tensor_add(Q2[:], t1q[:], t2q[:])

            # ---- attention ----
            GROUPS = [3] * 10 + [2]  # 32 t-tiles in PSUM-sized groups
            with (
                tc.tile_pool(name="pw", bufs=2, space="PSUM") as pwp,
                tc.tile_pool(name="po", bufs=1, space="PSUM") as pop,
                tc.tile_pool(name="pt", bufs=1, space="PSUM") as ptp,
                tc.tile_pool(name="esb", bufs=3) as esb,
                tc.tile_pool(name="osb", bufs=2) as osbp,
            ):
                for sc in range(NSC):
                    qs = Q2[:, sc * 512 : (sc + 1) * 512]
                    pso = pop.tile([65, 512], FP, tag="pso")
                    tt = 0
                    for gn in GROUPS:
                        psw = pwp.tile([128, 1536], FP, tag="psw")
                        et = esb.tile([128, 1536], BF, tag="et")
                        for j in range(gn):
                            nc.tensor.matmul(
                                psw[:, j * 512 : (j + 1) * 512],
                                lhsT=K2[:, (tt + j) * 128 : (tt + j + 1) * 128],
                                rhs=qs,
                                start=True, stop=True,
                            )
                        nc.scalar.activation(
                            et[:, : gn * 512], psw[:, : gn * 512], Exp, scale=SCALE
                        )
                        for j in range(gn):
                            nc.tensor.matmul(
                                pso[:],
                                lhsT=v_all[:, (tt + j) * 65 : (tt + j) * 65 + 65],
                                rhs=et[:, j * 512 : (j + 1) * 512],
                                start=(tt + j == 0), stop=(tt + j == NT - 1),
                            )
                        tt += gn

                    osb = osbp.tile([65, 512], FP, tag="osb")
                    nc.vector.tensor_copy(osb[:], pso[:])
                    out_sb = osbp.tile([128, 4 * HS], FP, tag="outsb")
                    for j in range(4):
                        pst = ptp.tile([128, 65], FP, tag="pst")
                        nc.tensor.transpose(
                            pst[:], osb[:, j * 128 : (j + 1) * 128], ident[0:65, 0:65]
                        )
                        zr = osbp.tile([128, 1], FP, tag="zr")
                        nc.vector.reciprocal(zr[:], pst[:, HS : HS + 1])
                        nc.vector.tensor_scalar_mul(
                            out_sb[:, j * HS : (j + 1) * HS], pst[:, 0:HS], zr[:]
                        )
                    nc.sync.dma_start(
                        out[sc * 512 : (sc + 1) * 512, :].rearrange(
                            "(j p) h -> p j h", p=128
                        ),
                        out_sb[:].rearrange("p (j h) -> p j h", j=4),
                    )

    _split_excess_waits(nc)
    _prog_cache["nc"] = nc
    return nc


def make_in_maps(x_image, x_text_emb, freqs_latex, freqs_img_x, freqs_img_y, Wk, Wq, Wv):
    """Host-side prep: transpose/cast activations, permute+transpose weights,
    build rope cos/sin tables in the permuted row layout."""
    perm = np.concatenate([np.arange(0, HS, 2), np.arange(1, HS, 2)])

    wk_dev = np.ascontiguousarray(np.asarray(Wk)[perm].T).astype(BF16)
    wq_dev = np.ascontiguousarray(np.asarray(Wq)[perm].T).astype(BF16)
    wv_dev = np.ascontiguousarray(np.asarray(Wv).T).astype(BF16)

    fx = np.asarray(freqs_img_x, dtype=np.float32)
    fy = np.asarray(freqs_img_y, dtype=np.float32)
    fl = np.asarray(freqs_latex, dtype=np.float32)
    ck_half = np.concatenate([fx[:, :, 0].T, fy[:, :, 0].T], axis=0)  # [32, TK]
    sk_half = np.concatenate([fx[:, :, 1].T, fy[:, :, 1].T], axis=0)
    cck = np.ascontiguousarray(np.concatenate([ck_half, ck_half], 0)).astype(BF16)
    ssk = np.ascontiguousarray(np.concatenate([-sk_half, sk_half], 0)).astype(BF16)
    cq_half = fl[:, :, 0].T  # [32, TQ]
    sq_half = fl[:, :, 1].T
    ccq = np.ascontiguousarray(np.concatenate([cq_half, cq_half], 0)).astype(BF16)
    ssq = np.ascontiguousarray(np.concatenate([-sq_half, sq_half], 0)).astype(BF16)

    xi = np.asarray(x_image, dtype=np.float32)
    xte = np.asarray(x_text_emb, dtype=np.float32)
    in_maps = []
    for b in range(N_CORES):
        in_maps.append(
            {
                "xt": np.ascontiguousarray(xi[b].T).astype(BF16),
                "xtt": np.ascontiguousarray(xte[b].T).astype(BF16),
                "wk": wk_dev, "wq": wq_dev, "wv": wv_dev,
                "cck": cck, "ssk": ssk, "ccq": ccq, "ssq": ssq,
            }
        )
    return in_maps


def kernel(x_image, x_text_emb, x_latex_mask, freqs_latex, freqs_img_x, freqs_img_y,
           Wk, Wq, Wv):
    del x_latex_mask  # unused in the reference
    from concourse.bass_utils import run_bass_kernel_spmd

    nc = build_program()
    in_maps = make_in_maps(
        x_image, x_text_emb, freqs_latex, freqs_img_x, freqs_img_y, Wk, Wq, Wv
    )
    res = run_bass_kernel_spmd(nc, in_maps, list(range(N_CORES)))
    return np.stack([res.results[b]["out"] for b in range(N_CORES)], axis=0)


# revision 9
# speedup vs baseline: 1.0747x; 1.0747x over previous
"""Trainium2 Bass kernel for nn_Cross_AttentionHead_withMask.

Cross-attention head: q = rope(x_text @ Wq.T), k = rope2d(x_image @ Wk.T),
v = x_image @ Wv.T, out = softmax(q k^T / sqrt(512)) v.
(x_latex_mask is accepted but unused — it is dead in the reference.)

Sharding: data-parallel over batch B=8, one batch per NeuronCore (8 cores).

Per-core device program (all matmuls bf16, accumulation/softmax stats fp32):
  - host ships x_image[b].T / x_text[b].T (bf16) so the contraction dim (C)
    lands on SBUF partitions without any on-device transposes
  - head dim is permuted to evens-then-odds so RoPE pairs become the row
    blocks [0:32] / [32:64]; rope = A*CC + partner(A)*SS (2 muls + 1 add)
  - scores computed transposed: weiT[t, s] = K2[:, t-tile].T @ Q2[:, s-chunk]
  - exp on ScalarE straight out of PSUM with the 1/sqrt(512) scale fused
  - attention-out: outT[h, s] += v_aug[t-tile].T @ expT, where v_aug carries
    a ones column so row 64 accumulates the softmax denominator for free
  - epilogue: PE-transpose [65, 128] -> [128, 65], per-partition reciprocal
    of the Z column, tensor_scalar multiply, DMA out
"""
import numpy as np
from contextlib import ExitStack

import ml_dtypes

B, TQ, TK = 8, 2048, 4096
DIM_IMG, DIM_TXT, HS = 512, 128, 64
N_CORES = 8
SCALE = float(DIM_IMG) ** -0.5  # reference scales by sqrt(image embed dim)

BF16 = ml_dtypes.bfloat16

_prog_cache = {}


def _patch_tile_drain():
    """This walrus build rejects a Drain carrying >1 sem wait; split the
    TileContext exit waits onto one-wait NoOps."""
    import concourse.tile as tile
    from concourse import mybir
    from concourse.vector_clock import ScopedClock

    if getattr(tile.TileContext, "_drain_patched", False):
        return

    def _drain_and_barrier(self, tick_clock, wait_clock):
        nc = self.nc
        nop = nc.sync.nop()
        wait_clock.add_sem_waits(nop.ins, ScopedClock({None: tick_clock.global_clock}))
        si = nop.ins.sync_info
        waits = list(si.on_wait) if si is not None else []
        if len(waits) > 1:
            nop.ins.sync_info = mybir.SyncInfo(on_wait=[waits[0]], on_update=[])
            for w in waits[1:]:
                extra = nc.sync.nop()
                extra.ins.sync_info = mybir.SyncInfo(on_wait=[w], on_update=[])
        nc.sync.drain()
        nc.all_engine_barrier()
        assert self.sems is not None
        popped = nc._tile_sem_poison_stack.pop()
        assert popped is self._sem_poison
        nc.clear_and_free_semaphores(list(self.sems.allocated().values()))
        nc.all_engine_barrier()

    tile.TileContext._drain_and_barrier = _drain_and_barrier
    tile.TileContext._drain_patched = True


def _split_excess_waits(nc):
    """This walrus build caps sem waits per instruction (1 for DMA/Drain-style
    control instructions, 2 for compute). Move excess waits onto same-engine
    NoOps inserted right before the offending instruction — the engine queue
    is FIFO, so blocking dispatch on the NoOp is semantically equivalent."""
    from concourse import mybir

    ctr = 0
    for fn in nc.m.functions:
        for b in fn.blocks:
            il = b.instructions
            new = []
            changed = False
            for inst in il:
                si = inst.sync_info
                waits = list(si.on_wait) if si is not None else []
                lim = 1
                if len(waits) > lim:
                    for w in waits[lim:]:
                        nop = mybir.InstNoOp(name=f"wsplit-{ctr}", ins=[], outs=[])
                        ctr += 1
                        nop.engine = inst.engine
                        nop.sync_info = mybir.SyncInfo(on_wait=[w], on_update=[])
                        new.append(nop)
                    inst.sync_info = mybir.SyncInfo(
                        on_wait=waits[:lim], on_update=list(si.on_update)
                    )
                    changed = True
                new.append(inst)
            if changed:
                b.instructions = new


def build_program(split_waits=True):
    """Build the single-core Bass program (same program runs SPMD on 8 cores)."""
    key = ("nc", split_waits)
    if key in _prog_cache:
        return _prog_cache[key]

    _patch_tile_drain()
    import concourse.bass as bass
    import concourse.tile as tile
    from concourse import mybir
    from concourse.masks import make_identity

    FP = mybir.dt.float32
    BF = mybir.dt.bfloat16

    nc = bass.Bass("TRN2", target_bir_lowering=False, debug=False)
    xt = nc.dram_tensor("xt", [DIM_IMG, TK], BF, kind="ExternalInput").ap()
    xtt = nc.dram_tensor("xtt", [DIM_TXT, TQ], BF, kind="ExternalInput").ap()
    wk = nc.dram_tensor("wk", [DIM_IMG, HS], BF, kind="ExternalInput").ap()
    wq = nc.dram_tensor("wq", [DIM_TXT, HS], BF, kind="ExternalInput").ap()
    wv = nc.dram_tensor("wv", [DIM_IMG, HS], BF, kind="ExternalInput").ap()
    cck = nc.dram_tensor("cck", [HS, TK], BF, kind="ExternalInput").ap()
    ssk = nc.dram_tensor("ssk", [HS, TK], BF, kind="ExternalInput").ap()
    ccq = nc.dram_tensor("ccq", [HS, TQ], BF, kind="ExternalInput").ap()
    ssq = nc.dram_tensor("ssq", [HS, TQ], BF, kind="ExternalInput").ap()
    out = nc.dram_tensor("out", [TQ, HS], FP, kind="ExternalOutput").ap()

    Exp = mybir.ActivationFunctionType.Exp
    NC4 = DIM_IMG // 128  # 4 c-chunks
    NT = TK // 128  # 32 t-tiles
    NSC = TQ // 512  # 4 s-chunks

    with tile.TileContext(nc) as tc:
        with ExitStack() as ctx:
            const = ctx.enter_context(tc.tile_pool(name="const", bufs=1))

            # ---- resident loads, ordered so q-proj can start immediately ----
            xtt_sb = const.tile([128, TQ], BF, tag="xtt")
            nc.sync.dma_start(xtt_sb[:], xtt[:])
            wq_sb = const.tile([128, HS], BF, tag="wq")
            nc.sync.dma_start(wq_sb[:], wq[:])
            ccq_sb = const.tile([HS, TQ], BF, tag="ccq")
            nc.sync.dma_start(ccq_sb[:], ccq[:])
            ssq_sb = const.tile([HS, TQ], BF, tag="ssq")
            nc.sync.dma_start(ssq_sb[:], ssq[:])
            wk_sb = const.tile([128, NC4 * HS], BF, tag="wk")
            nc.sync.dma_start(
                wk_sb[:].rearrange("p (a h) -> p a h", a=NC4),
                wk.rearrange("(a p) h -> p a h", p=128),
            )
            wv_sb = const.tile([128, NC4 * HS], BF, tag="wv")
            nc.sync.dma_start(
                wv_sb[:].rearrange("p (a h) -> p a h", a=NC4),
                wv.rearrange("(a p) h -> p a h", p=128),
            )
            # x_image.T: 4 MB; split across the SP and ACT HWDGE rings
            xt_sb = []
            for ci in range(NC4):
                t = const.tile([128, TK], BF, tag=f"xt{ci}")
                eng = nc.sync if ci % 2 == 0 else nc.gpsimd
                eng.dma_start(t[:], xt[ci * 128 : (ci + 1) * 128, :])
                xt_sb.append(t)
            cck_sb = const.tile([HS, TK], BF, tag="cck")
            nc.sync.dma_start(cck_sb[:], cck[:])
            ssk_sb = const.tile([HS, TK], BF, tag="ssk")
            nc.gpsimd.dma_start(ssk_sb[:], ssk[:])
            ident = const.tile([128, 128], FP, tag="ident")
            make_identity(nc, ident[:])

            kt_pre = const.tile([HS, TK], BF, tag="ktpre")
            qt_pre = const.tile([HS, TQ], BF, tag="qtpre")
            v_all = const.tile([128, NT * 65], BF, tag="vall")

            # ---- projections + rope, engine-split so PE never stalls:
            # PE: q-MMs -> k-MMs -> v-MMs back-to-back
            # DVE: q/k psum->sbuf copies + rope muls/adds
            # ACT: v psum->sbuf copies
            with tc.tile_pool(name="pp", bufs=3, space="PSUM") as pp:
                for j in range(TQ // 512):
                    ps = pp.tile([HS, 512], FP, tag="ps")
                    nc.tensor.matmul(
                        ps[:], lhsT=wq_sb[:], rhs=xtt_sb[:, j * 512 : (j + 1) * 512],
                        start=True, stop=True,
                    )
                    nc.vector.tensor_copy(qt_pre[:, j * 512 : (j + 1) * 512], ps[:])

                # q rope (DVE) while PE moves on to k-proj
                pq = const.tile([HS, TQ], BF, tag="pq")
                nc.sync.dma_start(pq[0:32, :], qt_pre[32:64, :])
                nc.sync.dma_start(pq[32:64, :], qt_pre[0:32, :])
                t1q = const.tile([HS, TQ], BF, tag="t1q")
                nc.vector.tensor_mul(t1q[:], qt_pre[:], ccq_sb[:])
                t2q = const.tile([HS, TQ], BF, tag="t2q")
                nc.vector.tensor_mul(t2q[:], pq[:], ssq_sb[:])
                Q2 = const.tile([HS, TQ], BF, tag="Q2")
                nc.vector.tensor_add(Q2[:], t1q[:], t2q[:])

                for j in range(TK // 512):
                    ps = pp.tile([HS, 512], FP, tag="ps")
                    for ci in range(NC4):
                        nc.tensor.matmul(
                            ps[:],
                            lhsT=wk_sb[:, ci * HS : (ci + 1) * HS],
                            rhs=xt_sb[ci][:, j * 512 : (j + 1) * 512],
                            start=(ci == 0), stop=(ci == NC4 - 1),
                        )
                    nc.vector.tensor_copy(kt_pre[:, j * 512 : (j + 1) * 512], ps[:])

                # k rope (DVE) while PE does v-proj
                pk = const.tile([HS, TK], BF, tag="pk")
                nc.sync.dma_start(pk[0:32, :], kt_pre[32:64, :])
                nc.sync.dma_start(pk[32:64, :], kt_pre[0:32, :])
                t1k = const.tile([HS, TK], BF, tag="t1k")
                nc.vector.tensor_mul(t1k[:], kt_pre[:], cck_sb[:])
                t2k = const.tile([HS, TK], BF, tag="t2k")
                nc.vector.tensor_mul(t2k[:], pk[:], ssk_sb[:])
                K2 = const.tile([HS, TK], BF, tag="K2")
                nc.vector.tensor_add(K2[:], t1k[:], t2k[:])

                for tt in range(NT):
                    ps = pp.tile([128, HS], FP, tag="psv")
                    for ci in range(NC4):
                        nc.tensor.matmul(
                            ps[:],
                            lhsT=xt_sb[ci][:, tt * 128 : (tt + 1) * 128],
                            rhs=wv_sb[:, ci * HS : (ci + 1) * HS],
                            start=(ci == 0), stop=(ci == NC4 - 1),
                        )
                    nc.scalar.copy(v_all[:, tt * 65 : tt * 65 + HS], ps[:])
            # ones column for the Z (softmax denominator) row
            nc.gpsimd.memset(v_all[:, HS :: 65], 1.0)

            # ---- attention ----
            GROUPS = [3] * 10 + [2]  # 32 t-tiles in PSUM-sized groups
            with (
                tc.tile_pool(name="pw", bufs=2, space="PSUM") as pwp,
                tc.tile_pool(name="po", bufs=2, space="PSUM") as pop,
                tc.tile_pool(name="esb", bufs=3) as esb,
                tc.tile_pool(name="osb", bufs=2) as osbp,
            ):
                for sc in range(NSC):
                    qs = Q2[:, sc * 512 : (sc + 1) * 512]
                    pso = pop.tile([65, 512], FP, tag="pso")
                    tt = 0
                    for gn in GROUPS:
                        psw = pwp.tile([128, 1536], FP, tag="psw")
                        et = esb.tile([128, 1536], BF, tag="et")
                        for j in range(gn):
                            nc.tensor.matmul(
                                psw[:, j * 512 : (j + 1) * 512],
                                lhsT=K2[:, (tt + j) * 128 : (tt + j + 1) * 128],
                                rhs=qs,
                                start=True, stop=True,
                            )
                        nc.scalar.activation(
                            et[:, : gn * 512], psw[:, : gn * 512], Exp, scale=SCALE
                        )
                        for j in range(gn):
                            nc.tensor.matmul(
                                pso[:],
                                lhsT=v_all[:, (tt + j) * 65 : (tt + j) * 65 + 65],
                                rhs=et[:, j * 512 : (j + 1) * 512],
                                start=(tt + j == 0), stop=(tt + j == NT - 1),
                            )
                        tt += gn

                    osb = osbp.tile([65, 512], FP, tag="osb")
                    nc.vector.tensor_copy(osb[:], pso[:])
                    out_sb = osbp.tile([128, 4 * HS], FP, tag="outsb")
                    for j in range(4):
                        pst = pwp.tile([128, 65], FP, tag="psw")
                        nc.tensor.transpose(
                            pst[:], osb[:, j * 128 : (j + 1) * 128], ident[0:65, 0:65]
                        )
                        zr = osbp.tile([128, 1], FP, tag="zr")
                        nc.vector.reciprocal(zr[:], pst[:, HS : HS + 1])
                        nc.vector.tensor_scalar_mul(
                            out_sb[:, j * HS : (j + 1) * HS], pst[:, 0:HS], zr[:]
                        )
                    nc.sync.dma_start(
                        out[sc * 512 : (sc + 1) * 512, :].rearrange(
                            "(j p) h -> p j h", p=128
                        ),
                        out_sb[:].rearrange("p (j h) -> p j h", j=4),
                    )

    if split_waits:
        _split_excess_waits(nc)
    _prog_cache[key] = nc
    return nc


def make_in_maps(x_image, x_text_emb, freqs_latex, freqs_img_x, freqs_img_y, Wk, Wq, Wv):
    """Host-side prep: transpose/cast activations, permute+transpose weights,
    build rope cos/sin tables in the permuted row layout."""
    perm = np.concatenate([np.arange(0, HS, 2), np.arange(1, HS, 2)])

    wk_dev = np.ascontiguousarray(np.asarray(Wk)[perm].T).astype(BF16)
    wq_dev = np.ascontiguousarray(np.asarray(Wq)[perm].T).astype(BF16)
    wv_dev = np.ascontiguousarray(np.asarray(Wv).T).astype(BF16)

    fx = np.asarray(freqs_img_x, dtype=np.float32)
    fy = np.asarray(freqs_img_y, dtype=np.float32)
    fl = np.asarray(freqs_latex, dtype=np.float32)
    ck_half = np.concatenate([fx[:, :, 0].T, fy[:, :, 0].T], axis=0)  # [32, TK]
    sk_half = np.concatenate([fx[:, :, 1].T, fy[:, :, 1].T], axis=0)
    cck = np.ascontiguousarray(np.concatenate([ck_half, ck_half], 0)).astype(BF16)
    ssk = np.ascontiguousarray(np.concatenate([-sk_half, sk_half], 0)).astype(BF16)
    cq_half = fl[:, :, 0].T  # [32, TQ]
    sq_half = fl[:, :, 1].T
    ccq = np.ascontiguousarray(np.concatenate([cq_half, cq_half], 0)).astype(BF16)
    ssq = np.ascontiguousarray(np.concatenate([-sq_half, sq_half], 0)).astype(BF16)

    xi = np.asarray(x_image, dtype=np.float32)
    xte = np.asarray(x_text_emb, dtype=np.float32)
    in_maps = []
    for b in range(N_CORES):
        in_maps.append(
            {
                "xt": np.ascontiguousarray(xi[b].T).astype(BF16),
                "xtt": np.ascontiguousarray(xte[b].T).astype(BF16),
                "wk": wk_dev, "wq": wq_dev, "wv": wv_dev,
                "cck": cck, "ssk": ssk, "ccq": ccq, "ssq": ssq,
            }
        )
    return in_maps


def kernel(x_image, x_text_emb, x_latex_mask, freqs_latex, freqs_img_x, freqs_img_y,
           Wk, Wq, Wv):
    del x_latex_mask  # unused in the reference
    from concourse.bass_utils import run_bass_kernel_spmd

    nc = build_program()
    in_maps = make_in_maps(
        x_image, x_text_emb, freqs_latex, freqs_img_x, freqs_img_y, Wk, Wq, Wv
    )
    res = run_bass_kernel_spmd(nc, in_maps, list(range(N_CORES)))
    return np.stack([res.results[b]["out"] for b in range(N_CORES)], axis=0)


# revision 10
# speedup vs baseline: 1.1575x; 1.0770x over previous
"""Trainium2 Bass kernel for nn_Cross_AttentionHead_withMask.

Cross-attention head: q = rope(x_text @ Wq.T), k = rope2d(x_image @ Wk.T),
v = x_image @ Wv.T, out = softmax(q k^T / sqrt(512)) v.
(x_latex_mask is accepted but unused — it is dead in the reference.)

Sharding: data-parallel over batch B=8, one batch per NeuronCore (8 cores).

Per-core device program (all matmuls bf16, accumulation/softmax stats fp32):
  - host ships x_image[b].T / x_text[b].T (bf16) so the contraction dim (C)
    lands on SBUF partitions without any on-device transposes
  - head dim is permuted to evens-then-odds so RoPE pairs become the row
    blocks [0:32] / [32:64]; rope = A*CC + partner(A)*SS (2 muls + 1 add)
  - scores computed transposed: weiT[t, s] = K2[:, t-tile].T @ Q2[:, s-chunk]
  - exp on ScalarE straight out of PSUM with the 1/sqrt(512) scale fused
  - attention-out: outT[h, s] += v_aug[t-tile].T @ expT, where v_aug carries
    a ones column so row 64 accumulates the softmax denominator for free
  - epilogue: PE-transpose [65, 128] -> [128, 65], per-partition reciprocal
    of the Z column, tensor_scalar multiply, DMA out
"""
import numpy as np
from contextlib import ExitStack

import ml_dtypes

B, TQ, TK = 8, 2048, 4096
DIM_IMG, DIM_TXT, HS = 512, 128, 64
N_CORES = 8
SCALE = float(DIM_IMG) ** -0.5  # reference scales by sqrt(image embed dim)

BF16 = ml_dtypes.bfloat16

_prog_cache = {}


def _patch_tile_drain():
    """This walrus build rejects a Drain carrying >1 sem wait; split the
    TileContext exit waits onto one-wait NoOps."""
    import concourse.tile as tile
    from concourse import mybir
    from concourse.vector_clock import ScopedClock

    if getattr(tile.TileContext, "_drain_patched", False):
        return

    def _drain_and_barrier(self, tick_clock, wait_clock):
        nc = self.nc
        nop = nc.sync.nop()
        wait_clock.add_sem_waits(nop.ins, ScopedClock({None: tick_clock.global_clock}))
        si = nop.ins.sync_info
        waits = list(si.on_wait) if si is not None else []
        if len(waits) > 1:
            nop.ins.sync_info = mybir.SyncInfo(on_wait=[waits[0]], on_update=[])
            for w in waits[1:]:
                extra = nc.sync.nop()
                extra.ins.sync_info = mybir.SyncInfo(on_wait=[w], on_update=[])
        nc.sync.drain()
        nc.all_engine_barrier()
        assert self.sems is not None
        popped = nc._tile_sem_poison_stack.pop()
        assert popped is self._sem_poison
        nc.clear_and_free_semaphores(list(self.sems.allocated().values()))
        nc.all_engine_barrier()

    tile.TileContext._drain_and_barrier = _drain_and_barrier
    tile.TileContext._drain_patched = True


def _split_excess_waits(nc):
    """This walrus build caps sem waits per instruction (1 for DMA/Drain-style
    control instructions, 2 for compute). Move excess waits onto same-engine
    NoOps inserted right before the offending instruction — the engine queue
    is FIFO, so blocking dispatch on the NoOp is semantically equivalent."""
    from concourse import mybir

    ctr = 0
    for fn in nc.m.functions:
        for b in fn.blocks:
            il = b.instructions
            new = []
            changed = False
            for inst in il:
                si = inst.sync_info
                waits = list(si.on_wait) if si is not None else []
                lim = 1
                if len(waits) > lim:
                    for w in waits[lim:]:
                        nop = mybir.InstNoOp(name=f"wsplit-{ctr}", ins=[], outs=[])
                        ctr += 1
                        nop.engine = inst.engine
                        nop.sync_info = mybir.SyncInfo(on_wait=[w], on_update=[])
                        new.append(nop)
                    inst.sync_info = mybir.SyncInfo(
                        on_wait=waits[:lim], on_update=list(si.on_update)
                    )
                    changed = True
                new.append(inst)
            if changed:
                b.instructions = new


def build_program(split_waits=True):
    """Build the single-core Bass program (same program runs SPMD on 8 cores)."""
    key = ("nc", split_waits)
    if key in _prog_cache:
        return _prog_cache[key]

    _patch_tile_drain()
    import concourse.bass as bass
    import concourse.tile as tile
    from concourse import mybir
    from concourse.masks import make_identity

    FP = mybir.dt.float32
    BF = mybir.dt.bfloat16

    nc = bass.Bass("TRN2", target_bir_lowering=False, debug=False)
    xt = nc.dram_tensor("xt", [DIM_IMG, TK], BF, kind="ExternalInput").ap()
    xtt = nc.dram_tensor("xtt", [DIM_TXT, TQ], BF, kind="ExternalInput").ap()
    wk = nc.dram_tensor("wk", [DIM_IMG, HS], BF, kind="ExternalInput").ap()
    wq = nc.dram_tensor("wq", [DIM_TXT, HS], BF, kind="ExternalInput").ap()
    wv = nc.dram_tensor("wv", [DIM_IMG, HS], BF, kind="ExternalInput").ap()
    cck = nc.dram_tensor("cck", [HS, TK], BF, kind="ExternalInput").ap()
    ssk = nc.dram_tensor("ssk", [HS, TK], BF, kind="ExternalInput").ap()
    ccq = nc.dram_tensor("ccq", [HS, TQ], BF, kind="ExternalInput").ap()
    ssq = nc.dram_tensor("ssq", [HS, TQ], BF, kind="ExternalInput").ap()
    out = nc.dram_tensor("out", [TQ, HS], FP, kind="ExternalOutput").ap()

    Exp = mybir.ActivationFunctionType.Exp
    NC4 = DIM_IMG // 128  # 4 c-chunks
    NT = TK // 128  # 32 t-tiles
    NSC = TQ // 512  # 4 s-chunks

    with tile.TileContext(nc) as tc:
        with ExitStack() as ctx:
            const = ctx.enter_context(tc.tile_pool(name="const", bufs=1))

            # ---- resident loads, ordered so q-proj can start immediately ----
            xtt_sb = const.tile([128, TQ], BF, tag="xtt")
            nc.sync.dma_start(xtt_sb[:], xtt[:])
            wq_sb = const.tile([128, HS], BF, tag="wq")
            nc.sync.dma_start(wq_sb[:], wq[:])
            ccq_sb = const.tile([HS, TQ], BF, tag="ccq")
            nc.sync.dma_start(ccq_sb[:], ccq[:])
            ssq_sb = const.tile([HS, TQ], BF, tag="ssq")
            nc.sync.dma_start(ssq_sb[:], ssq[:])
            wk_sb = const.tile([128, NC4 * HS], BF, tag="wk")
            nc.sync.dma_start(
                wk_sb[:].rearrange("p (a h) -> p a h", a=NC4),
                wk.rearrange("(a p) h -> p a h", p=128),
            )
            wv_sb = const.tile([128, NC4 * HS], BF, tag="wv")
            nc.sync.dma_start(
                wv_sb[:].rearrange("p (a h) -> p a h", a=NC4),
                wv.rearrange("(a p) h -> p a h", p=128),
            )
            # x_image.T: 4 MB; split across the SP and ACT HWDGE rings
            xt_sb = []
            for ci in range(NC4):
                t = const.tile([128, TK], BF, tag=f"xt{ci}")
                eng = nc.sync if ci % 2 == 0 else nc.gpsimd
                eng.dma_start(t[:], xt[ci * 128 : (ci + 1) * 128, :])
                xt_sb.append(t)
            cck_sb = const.tile([HS, TK], BF, tag="cck")
            nc.sync.dma_start(cck_sb[:], cck[:])
            ssk_sb = const.tile([HS, TK], BF, tag="ssk")
            nc.gpsimd.dma_start(ssk_sb[:], ssk[:])
            ident = const.tile([128, 128], FP, tag="ident")
            make_identity(nc, ident[:])

            kt_pre = const.tile([HS, TK], BF, tag="ktpre")
            qt_pre = const.tile([HS, TQ], BF, tag="qtpre")
            v_all = const.tile([128, NT * 65], BF, tag="vall")

            # ---- projections + rope, engine-split so PE never stalls:
            # PE: q-MMs -> k-MMs -> v-MMs back-to-back
            # DVE: q/k psum->sbuf copies + rope muls/adds
            # ACT: v psum->sbuf copies
            with tc.tile_pool(name="pp", bufs=3, space="PSUM") as pp:
                for j in range(TQ // 512):
                    ps = pp.tile([HS, 512], FP, tag="ps")
                    nc.tensor.matmul(
                        ps[:], lhsT=wq_sb[:], rhs=xtt_sb[:, j * 512 : (j + 1) * 512],
                        start=True, stop=True,
                    )
                    nc.vector.tensor_copy(qt_pre[:, j * 512 : (j + 1) * 512], ps[:])

                # q rope (DVE) while PE moves on to k-proj
                pq = const.tile([HS, TQ], BF, tag="pq")
                nc.sync.dma_start(pq[0:32, :], qt_pre[32:64, :])
                nc.sync.dma_start(pq[32:64, :], qt_pre[0:32, :])
                t1q = const.tile([HS, TQ], BF, tag="t1q")
                nc.vector.tensor_mul(t1q[:], qt_pre[:], ccq_sb[:])
                t2q = const.tile([HS, TQ], BF, tag="t2q")
                nc.vector.tensor_mul(t2q[:], pq[:], ssq_sb[:])
                Q2 = const.tile([HS, TQ], BF, tag="Q2")
                nc.vector.tensor_add(Q2[:], t1q[:], t2q[:])

                for j in range(TK // 512):
                    ps = pp.tile([HS, 512], FP, tag="ps")
                    for ci in range(NC4):
                        nc.tensor.matmul(
                            ps[:],
                            lhsT=wk_sb[:, ci * HS : (ci + 1) * HS],
                            rhs=xt_sb[ci][:, j * 512 : (j + 1) * 512],
                            start=(ci == 0), stop=(ci == NC4 - 1),
                        )
                    nc.vector.tensor_copy(kt_pre[:, j * 512 : (j + 1) * 512], ps[:])

                # k rope (DVE) while PE does v-proj, chunked so the first
                # half of K2 is ready before v-proj finishes
                pk = const.tile([HS, TK], BF, tag="pk")
                t1k = const.tile([HS, TK], BF, tag="t1k")
                t2k = const.tile([HS, TK], BF, tag="t2k")
                K2 = const.tile([HS, TK], BF, tag="K2")
                for h in range(2):
                    cs = slice(h * (TK // 2), (h + 1) * (TK // 2))
                    nc.sync.dma_start(pk[0:32, cs], kt_pre[32:64, cs])
                    nc.sync.dma_start(pk[32:64, cs], kt_pre[0:32, cs])
                    nc.vector.tensor_mul(t1k[:, cs], kt_pre[:, cs], cck_sb[:, cs])
                    nc.vector.tensor_mul(t2k[:, cs], pk[:, cs], ssk_sb[:, cs])
                    nc.vector.tensor_add(K2[:, cs], t1k[:, cs], t2k[:, cs])

                for tt in range(NT):
                    ps = pp.tile([128, HS], FP, tag="psv")
                    for ci in range(NC4):
                        nc.tensor.matmul(
                            ps[:],
                            lhsT=xt_sb[ci][:, tt * 128 : (tt + 1) * 128],
                            rhs=wv_sb[:, ci * HS : (ci + 1) * HS],
                            start=(ci == 0), stop=(ci == NC4 - 1),
                        )
                    nc.scalar.copy(v_all[:, tt * 65 : tt * 65 + HS], ps[:])
            # ones column for the Z (softmax denominator) row
            nc.gpsimd.memset(v_all[:, HS :: 65], 1.0)

            # ---- attention ----
            GROUPS = [3] * 10 + [2]  # 32 t-tiles in PSUM-sized groups
            with (
                tc.tile_pool(name="pw", bufs=2, space="PSUM") as pwp,
                tc.tile_pool(name="po", bufs=2, space="PSUM") as pop,
                tc.tile_pool(name="esb", bufs=3) as esb,
                tc.tile_pool(name="osb", bufs=2) as osbp,
            ):
                for sc in range(NSC):
                    qs = Q2[:, sc * 512 : (sc + 1) * 512]
                    pso = pop.tile([65, 512], FP, tag="pso")

                    def att_group(pend):
                        pet, pgn, ptt = pend
                        for j in range(pgn):
                            nc.tensor.matmul(
                                pso[:],
                                lhsT=v_all[:, (ptt + j) * 65 : (ptt + j) * 65 + 65],
                                rhs=pet[:, j * 512 : (j + 1) * 512],
                                start=(ptt + j == 0), stop=(ptt + j == NT - 1),
                            )

                    tt = 0
                    pend = None
                    for gn in GROUPS:
                        psw = pwp.tile([128, 1536], FP, tag="psw")
                        et = esb.tile([128, 1536], BF, tag="et")
                        for j in range(gn):
                            nc.tensor.matmul(
                                psw[:, j * 512 : (j + 1) * 512],
                                lhsT=K2[:, (tt + j) * 128 : (tt + j + 1) * 128],
                                rhs=qs,
                                start=True, stop=True,
                            )
                        nc.scalar.activation(
                            et[:, : gn * 512], psw[:, : gn * 512], Exp, scale=SCALE
                        )
                        # att-out MMs for the PREVIOUS group: its exp output is
                        # ready, so PE never stalls on ScalarE
                        if pend is not None:
                            att_group(pend)
                        pend = (et, gn, tt)
                        tt += gn
                    att_group(pend)

                    osb = osbp.tile([65, 512], FP, tag="osb")
                    nc.vector.tensor_copy(osb[:], pso[:])
                    out_sb = osbp.tile([128, 4 * HS], FP, tag="outsb")
                    for j in range(4):
                        pst = pwp.tile([128, 65], FP, tag="psw")
                        nc.tensor.transpose(
                            pst[:], osb[:, j * 128 : (j + 1) * 128], ident[0:65, 0:65]
                        )
                        zr = osbp.tile([128, 1], FP, tag="zr")
                        nc.vector.reciprocal(zr[:], pst[:, HS : HS + 1])
                        nc.vector.tensor_scalar_mul(
                            out_sb[:, j * HS : (j + 1) * HS], pst[:, 0:HS], zr[:]
                        )
                    nc.sync.dma_start(
                        out[sc * 512 : (sc + 1) * 512, :].rearrange(
                            "(j p) h -> p j h", p=128
                        ),
                        out_sb[:].rearrange("p (j h) -> p j h", j=4),
                    )

    if split_waits:
        _split_excess_waits(nc)
    _prog_cache[key] = nc
    return nc


def make_in_maps(x_image, x_text_emb, freqs_latex, freqs_img_x, freqs_img_y, Wk, Wq, Wv):
    """Host-side prep: transpose/cast activations, permute+transpose weights,
    build rope cos/sin tables in the permuted row layout."""
    perm = np.concatenate([np.arange(0, HS, 2), np.arange(1, HS, 2)])

    wk_dev = np.ascontiguousarray(np.asarray(Wk)[perm].T).astype(BF16)
    wq_dev = np.ascontiguousarray(np.asarray(Wq)[perm].T).astype(BF16)
    wv_dev = np.ascontiguousarray(np.asarray(Wv).T).astype(BF16)

    fx = np.asarray(freqs_img_x, dtype=np.float32)
    fy = np.asarray(freqs_img_y, dtype=np.float32)
    fl = np.asarray(freqs_latex, dtype=np.float32)
    ck_half = np.concatenate([fx[:, :, 0].T, fy[:, :, 0].T], axis=0)  # [32, TK]
    sk_half = np.concatenate([fx[:, :, 1].T, fy[:, :, 1].T], axis=0)
    cck = np.ascontiguousarray(np.concatenate([ck_half, ck_half], 0)).astype(BF16)
    ssk = np.ascontiguousarray(np.concatenate([-sk_half, sk_half], 0)).astype(BF16)
    cq_half = fl[:, :, 0].T  # [32, TQ]
    sq_half = fl[:, :, 1].T
    ccq = np.ascontiguousarray(np.concatenate([cq_half, cq_half], 0)).astype(BF16)
    ssq = np.ascontiguousarray(np.concatenate([-sq_half, sq_half], 0)).astype(BF16)

    xi = np.asarray(x_image, dtype=np.float32)
    xte = np.asarray(x_text_emb, dtype=np.float32)
    in_maps = []
    for b in range(N_CORES):
        in_maps.append(
            {
                "xt": np.ascontiguousarray(xi[b].T).astype(BF16),
                "xtt": np.ascontiguousarray(xte[b].T).astype(BF16),
                "wk": wk_dev, "wq": wq_dev, "wv": wv_dev,
                "cck": cck, "ssk": ssk, "ccq": ccq, "ssq": ssq,
            }
        )
    return in_maps


def kernel(x_image, x_text_emb, x_latex_mask, freqs_latex, freqs_img_x, freqs_img_y,
           Wk, Wq, Wv):
    del x_latex_mask  # unused in the reference
    from concourse.bass_utils import run_bass_kernel_spmd

    nc = build_program()
    in_maps = make_in_maps(
        x_image, x_text_emb, freqs_latex, freqs_img_x, freqs_img_y, Wk, Wq, Wv
    )
    res = run_bass_kernel_spmd(nc, in_maps, list(range(N_CORES)))
    return np.stack([res.results[b]["out"] for b in range(N_CORES)], axis=0)
